# revision 23
# baseline (speedup 1.0000x reference)
"""Trainium2 Bass kernel for nn_CropModule: per-sample crop + bilinear resize.

Contract: kernel(img [128,3,480,480] f32, box [128,4] f32) -> [128, 150528] f32.

Strategy (pure data parallel, 16 slots per NeuronCore across 8 cores):
  * The Bass program is specialized to the actual crop sizes at runtime
    (kernel() compiles after seeing the inputs; recompiled whenever the
    crop geometry changes).  The 128 samples are grouped into 16 slots of
    8 (one per core, SPMD) by similar (crop_h, crop_w); each slot's DMA
    rectangles and matmul access patterns are sized to the slot maxima
    CH=max(ch), CW=max(cw) instead of the 240x240 worst case.  That cuts
    HBM traffic ~35% and tensor cycles ~12%, which matters because the
    kernel is DMA-bound (input reads sustain only ~210 GB/s while compute
    runs).
  * Host computes, per sample, the CHxCW crop window and two bilinear
    selection tables RyT [CH,224] / RxT [CW,224] (bilinear weight
    w[i,o] = relu(1 - |i - src(o)|), identical to the reference), packed
    per slot as [R, ty kcy*224 | win (c,yc,x) 3*kcy*CW | tx kcx*224]
    bf16 with R = min(120, max(CH, CW)) partition rows.
  * Device, per slot and channel, two accumulating bf16 matmul passes
    (f32 PSUM accumulate):
        mid[x, oy] = sum_y W[y, x] * RyT[y, oy]      (pass V)
        out[oy, ox] = sum_x mid[x, oy] * RxT[x, ox]  (pass H)
    Pass H of slot u-1 is emitted after the full pass V of slot u so the
    tensor engine never waits on the PSUM->SBUF cast of mid.
  * Host unpacks to [B, 3, 224, 224] f32 (undoing the slot permutation).
    bf16 end-to-end rel err vs the f32 reference is ~8e-3 (gate 2e-2).
"""
from contextlib import ExitStack

import ml_dtypes
import numpy as np

import concourse.bass as bass
import concourse.mybir as mybir
import concourse.tile as tile
from concourse.bass_utils import run_bass_kernel_spmd
from concourse.vector_clock import ScopedClock

IMG = 480
OUT = 224
BATCH = 128
N_CORES = 8
NSAMP = BATCH // N_CORES

OUT_COLS = 3 * 2 * OUT        # 1344: (c, oc, ox)
OUT_BATCH = 2                 # samples per output store group

F32 = mybir.dt.float32
BF16 = mybir.dt.bfloat16
BF16_NP = ml_dtypes.bfloat16


# ------------------------------------------------------------------ geometry

def _crop_dims(box):
    """-> xa, ya, ch, cw (int32 arrays [B])."""
    b = box.astype(np.float32) * np.float32(IMG)
    xa = np.trunc(b[:, 0] - np.float32(0.5) * b[:, 2]).astype(np.int32)
    ya = np.trunc(b[:, 1] - np.float32(0.5) * b[:, 3]).astype(np.int32)
    xb = np.trunc(b[:, 0] + np.float32(0.5) * b[:, 2]).astype(np.int32)
    yb = np.trunc(b[:, 1] + np.float32(0.5) * b[:, 3]).astype(np.int32)
    return xa, ya, (yb - ya).astype(np.int32), (xb - xa).astype(np.int32)


def _plan(box):
    """Group the B samples into NSAMP slots of N_CORES cores each.

    Samples with similar (ch, cw) share a slot so the per-slot maxima are
    tight.  Returns (groups [NSAMP][N_CORES] sample idx, slots [(CH,CW)]).
    """
    _, _, ch, cw = _crop_dims(box)
    best = None
    for nblk in (2, 4, 8):
        idx = np.argsort(ch, kind="stable")
        groups = []
        bs = len(idx) // nblk
        for blk in range(nblk):
            bi = idx[blk * bs:(blk + 1) * bs]
            bi = bi[np.argsort(cw[bi], kind="stable")]
            for g in range(bs // N_CORES):
                groups.append(bi[g * N_CORES:(g + 1) * N_CORES])
        slots = [(int(ch[g].max()), int(cw[g].max())) for g in groups]
        cost = sum(g.R * g.cols for g in (_Geom(*s) for s in slots))
        if best is None or cost < best[0]:
            best = (cost, groups, slots)
    _, groups, slots = best
    order = list(np.argsort([CH * CW for CH, CW in slots], kind="stable"))
    # the slot with the least pass-H work goes last: the tail between the
    # final matmul and the final store is dominated by H + copies
    hcost = [(_Geom(*slots[i]).kcx, slots[i][0] * slots[i][1]) for i in order]
    last = order.pop(int(np.argmin([h[0] * 1000000 + h[1] for h in hcost])))
    order.append(last)
    groups = [groups[i] for i in order]
    slots = [slots[i] for i in order]
    return groups, slots


def _ceil_div(a, b):
    return -(-a // b)


class _Geom:
    """Per-slot packed-block geometry.  Contraction chunks are split
    EVENLY (not at 120/128) so both chunks have the same row count and the
    packed rectangle has no row padding."""

    def __init__(self, CH, CW):
        self.CH, self.CW = CH, CW
        self.kcy = _ceil_div(CH, 128)
        self.kcx = _ceil_div(CW, 128)
        ry0 = _ceil_div(CH, self.kcy)
        rx0 = _ceil_div(CW, self.kcx)
        self.ry = (ry0, CH - ry0)
        self.rx = (rx0, CW - rx0)
        # a DMA's descriptors are sprayed over (largest divisor of the
        # partition-row count that is <= 16) DMA engines: 120 rows -> 15
        # engines, 82=2*41 -> 2, primes -> 1.  Pad the shipped rows to a
        # multiple of 16 so every input DMA uses all 16 engines.
        self.R = _ceil_div(max(ry0, rx0), 16) * 16
        self.woff = self.kcy * OUT                       # window cols base
        self.txoff = self.woff + 3 * self.kcy * CW       # RxT cols base
        self.cols = self.txoff + self.kcx * OUT


# ---------------------------------------------------------------- host prep

def _axis_tab(ca, cn, w0, rows):
    """Bilinear table [rows, OUT] for one axis of one sample (f32 math as
    in the reference): tab[i, o] = relu(1 - |i - (s(o) + ca - w0)|)."""
    o = np.arange(OUT, dtype=np.float32)
    cnf = np.float32(cn)
    s = np.clip((o + np.float32(0.5)) * cnf / np.float32(OUT)
                - np.float32(0.5), np.float32(0.0), cnf - np.float32(1.0))
    s = s + np.float32(ca - w0)
    i = np.arange(rows, dtype=np.float32)[:, None]
    return np.maximum(np.float32(0.0),
                      np.float32(1.0) - np.abs(i - s[None, :]))


def _prep(img, box, groups, slots):
    """-> per-core input blocks [N_CORES][120, tot_cols] bf16."""
    geos = [_Geom(*s) for s in slots]
    blocks = [{} for _ in range(N_CORES)]
    xa_, ya_, ch_, cw_ = _crop_dims(box)
    for si, (g, geo) in enumerate(zip(groups, geos)):
        CH, CW = geo.CH, geo.CW
        ry0, rx0 = geo.ry[0], geo.rx[0]
        for core, smp in enumerate(g):
            blk = np.zeros((geo.R, geo.cols), dtype=BF16_NP)
            xa, ya = int(xa_[smp]), int(ya_[smp])
            ch, cw = int(ch_[smp]), int(cw_[smp])
            wy0 = min(ya, IMG - CH)
            wx0 = min(xa, IMG - CW)
            ty = _axis_tab(ya, ch, wy0, CH).astype(BF16_NP)
            tx = _axis_tab(xa, cw, wx0, CW).astype(BF16_NP)
            for yc in range(geo.kcy):
                r = geo.ry[yc]
                blk[0:r, yc * OUT:yc * OUT + OUT] = ty[yc * ry0:yc * ry0 + r]
            for xc in range(geo.kcx):
                r = geo.rx[xc]
                c0 = geo.txoff + xc * OUT
                blk[0:r, c0:c0 + OUT] = tx[xc * rx0:xc * rx0 + r]
            win = img[smp, :, wy0:wy0 + CH, wx0:wx0 + CW].astype(BF16_NP)
            for c in range(3):
                for yc in range(geo.kcy):
                    r = geo.ry[yc]
                    c0 = geo.woff + (c * geo.kcy + yc) * CW
                    blk[0:r, c0:c0 + CW] = win[c, yc * ry0:yc * ry0 + r]
            blocks[core][f"inp{si}"] = blk
    return blocks


def _unpack(parts, groups):
    """parts [N_CORES][112, NSAMP*OUT_COLS] bf16 -> [B, 3*224*224] f32."""
    full = np.empty((BATCH, 3, OUT, 2, 112), dtype=np.float32)
    for core in range(N_CORES):
        o = parts[core].reshape(112, NSAMP, 2, 3, OUT)
        o = o.transpose(1, 3, 4, 2, 0).astype(np.float32)  # [slot,c,oy,oc,p]
        for g, grp in enumerate(groups):
            full[grp[core]] = o[g]
    return np.ascontiguousarray(full).reshape(BATCH, -1)


# ------------------------------------------------- walrus wait-limit fixups

class _SplitDrainTileContext(tile.TileContext):
    """The walrus build here rejects instructions carrying several sync
    waits; re-emit the kernel-tail drain's waits as single-wait NoOps,
    spread round-robin across engines so they retire in parallel."""

    def _drain_and_barrier(self, tick_clock, wait_clock):
        nc = self.nc
        probe = nc.sync.nop(nofuse=True, hint="drain_wait_probe")
        wait_clock.add_sem_waits(
            probe.ins, ScopedClock({None: tick_clock.global_clock}))
        si = probe.ins.sync_info
        waits = list(si.on_wait) if si is not None else []
        if si is not None:
            si.on_wait = waits[:1]
        wait_engines = [nc.sync, nc.scalar, nc.vector, nc.tensor]
        for i, w in enumerate(waits[1:]):
            eng = wait_engines[i % len(wait_engines)]
            n = eng.nop(nofuse=True, hint="drain_wait_split")
            n.ins.sync_info = mybir.SyncInfo(on_wait=[w], on_update=[])
        nc.sync.drain()

        nc.all_engine_barrier()
        assert self.sems is not None
        popped = nc._tile_sem_poison_stack.pop()
        assert popped is self._sem_poison
        # Skip the gpsimd range clear + second barrier: the runtime
        # re-initializes kernel semaphores at exec start (validated by
        # back-to-back execs of the same NEFF), and the Q7 wakeup for the
        # clear costs ~5 us of pure tail.


def _split_sync_waits(nc, max_waits=1):
    ctr = 0
    for fn in nc.m.functions:
        for blk in fn.blocks:
            out = []
            for inst in blk.instructions:
                si = getattr(inst, "sync_info", None)
                waits = list(si.on_wait) if si is not None and si.on_wait else []
                if len(waits) > max_waits:
                    for w in waits[:-max_waits]:
                        ctr += 1
                        out.append(mybir.InstNoOp(
                            name=f"wsplit_{ctr}",
                            engine=inst.engine,
                            ins=[], outs=[],
                            sync_info=mybir.SyncInfo(on_wait=[w], on_update=[])))
                    si.on_wait = waits[-max_waits:]
                out.append(inst)
            blk.instructions = out


# ------------------------------------------------------------ device kernel

def build_kernel(slots, n_cores=N_CORES):
    nsamp = len(slots)
    geos = [_Geom(*s) for s in slots]

    nc = bass.Bass("TRN2", target_bir_lowering=False, debug=False,
                   num_devices=n_cores)
    # one DRAM tensor per slot: DMAs that cover a tensor's full partition
    # dim are sprayed across all 16 DMA engines, partition-sliced ones
    # serialize onto a single engine
    inps = [nc.dram_tensor(f"inp{s}", [geos[s].R, geos[s].cols], BF16,
                           kind="ExternalInput") for s in range(nsamp)]
    out = nc.dram_tensor("out", [112, nsamp * OUT_COLS], BF16,
                         kind="ExternalOutput")

    with _SplitDrainTileContext(nc) as tc, ExitStack() as ctx:
        inpp = ctx.enter_context(tc.tile_pool(name="inpp", bufs=nsamp))
        midp = ctx.enter_context(tc.tile_pool(name="midp", bufs=3))
        outp = ctx.enter_context(tc.tile_pool(name="outp", bufs=3))
        midps = ctx.enter_context(tc.tile_pool(name="midps", bufs=3, space="PSUM"))
        abps = ctx.enter_context(tc.tile_pool(name="abps", bufs=3, space="PSUM"))
        cps = ctx.enter_context(tc.tile_pool(name="cps", bufs=2, space="PSUM"))

        pending = None  # (sb, geo, mid3, out_sb, s)

        def emit_h(u):
            sb, geo, mid3, out_sb, s = u
            # mid3 layout (xc, c, oy); moving operand caps at 512 elements,
            # so stream channels (0,1) fused (448 cols) and channel 2 alone
            final = s == nsamp - 1
            for oc in range(2):
                ps_ab = abps.tile([112, 2 * OUT], F32)
                ps_c = cps.tile([112, OUT], F32)
                for xc in range(geo.kcx):
                    xw = geo.rx[xc]
                    t0 = geo.txoff + xc * OUT + oc * 112
                    lhsT = sb[0:xw, t0:t0 + 112]
                    nc.tensor.matmul(
                        ps_ab[:], lhsT=lhsT,
                        rhs=mid3[0:xw, xc * 3 * OUT:xc * 3 * OUT + 2 * OUT],
                        start=(xc == 0), stop=(xc == geo.kcx - 1))
                    nc.tensor.matmul(
                        ps_c[:], lhsT=lhsT,
                        rhs=mid3[0:xw, xc * 3 * OUT + 2 * OUT:
                                 (xc + 1) * 3 * OUT],
                        start=(xc == 0), stop=(xc == geo.kcx - 1))
                ob = (s % OUT_BATCH) * OUT_COLS + oc * 3 * OUT
                if final and oc == 1:
                    # last slot: copy on the idle vector engine (parallel
                    # with scalar's oc=0 copy) and store each half as soon
                    # as it is ready, shrinking the exposed final transfer
                    nc.vector.tensor_copy(out_sb[:, ob:ob + 2 * OUT], ps_ab[:])
                    nc.vector.tensor_copy(
                        out_sb[:, ob + 2 * OUT:ob + 3 * OUT], ps_c[:])
                else:
                    nc.scalar.copy(out=out_sb[:, ob:ob + 2 * OUT], in_=ps_ab[:])
                    nc.scalar.copy(out=out_sb[:, ob + 2 * OUT:ob + 3 * OUT],
                                   in_=ps_c[:])
                if final:
                    # issue the two final stores from different engines so
                    # they go out in parallel
                    eng = nc.sync if oc == 0 else nc.scalar
                    eng.dma_start(
                        out.ap()[:, s * OUT_COLS + oc * 3 * OUT:
                                 s * OUT_COLS + (oc + 1) * 3 * OUT],
                        out_sb[:, ob:ob + 3 * OUT])
            if s == nsamp - 2:
                # store the penultimate slot alone so the final exposed
                # transfer after the last compute is as small as possible
                nc.sync.dma_start(
                    out.ap()[:, s * OUT_COLS:(s + 1) * OUT_COLS],
                    out_sb[:, 0:OUT_COLS])
            elif s != nsamp - 1 and s % OUT_BATCH == OUT_BATCH - 1:
                g0 = s - OUT_BATCH + 1
                nc.scalar.dma_start(
                    out.ap()[:, g0 * OUT_COLS:(g0 + OUT_BATCH) * OUT_COLS],
                    out_sb[:])

        out_sb = None
        for s in range(nsamp):
            geo = geos[s]
            sb = inpp.tile([geo.R, geo.cols], BF16)
            if s == 0:
                # the very first load lands as [RyT + channel-0 window]
                # (all that V(0).c0 gates on), then the rest
                c1 = geo.woff + geo.kcy * geo.CW
                nc.sync.dma_start(sb[:, 0:c1], inps[s].ap()[:, 0:c1])
                nc.sync.dma_start(sb[:, c1:geo.txoff],
                                  inps[s].ap()[:, c1:geo.txoff])
                nc.sync.dma_start(sb[:, geo.txoff:geo.cols],
                                  inps[s].ap()[:, geo.txoff:geo.cols])
            elif s == 1:
                nc.sync.dma_start(sb[:, 0:geo.txoff],
                                  inps[s].ap()[:, 0:geo.txoff])
                nc.sync.dma_start(sb[:, geo.txoff:geo.cols],
                                  inps[s].ap()[:, geo.txoff:geo.cols])
            else:
                nc.sync.dma_start(sb[:], inps[s].ap()[:])
            if s % OUT_BATCH == 0:
                out_sb = outp.tile([112, OUT_BATCH * OUT_COLS], BF16)
            mid3 = midp.tile([128, geo.kcx * 3 * OUT], BF16)
            for c in range(3):
                mid_ps = midps.tile([128, 2 * OUT], F32)
                for xc in range(geo.kcx):
                    xw = geo.rx[xc]
                    for yc in range(geo.kcy):
                        rr = geo.ry[yc]
                        w0 = geo.woff + (c * geo.kcy + yc) * geo.CW \
                            + xc * geo.rx[0]
                        nc.tensor.matmul(
                            mid_ps[0:xw, xc * OUT:(xc + 1) * OUT],
                            lhsT=sb[0:rr, w0:w0 + xw],
                            rhs=sb[0:rr, yc * OUT:(yc + 1) * OUT],
                            start=(yc == 0), stop=(yc == geo.kcy - 1))
                # mid_ps is (xc, oy); scatter the xc halves into the
                # (xc, c, oy) layout of mid3 with one strided-dest copy
                mid3v = mid3[:].rearrange(
                    "p (a c o) -> p a c o", a=geo.kcx, c=3, o=OUT)[:, :, c, :]
                nc.vector.tensor_copy(
                    mid3v, mid_ps[:, 0:geo.kcx * OUT].rearrange(
                        "p (a o) -> p a o", a=geo.kcx, o=OUT))
            # emit H(s-1) only now: the full V(s) sits between the c2 cast
            # of slot s-1 (vector) and the H matmuls that consume it, so
            # the tensor engine never waits on the cast
            if pending is not None:
                emit_h(pending)
            pending = (sb, geo, mid3, out_sb, s)
        emit_h(pending)
    _split_sync_waits(nc)
    return nc


_NC_CACHE = {}


def _run(img, box, trace=False, trace_kwargs=None):
    img = np.asarray(img, dtype=np.float32)
    box = np.asarray(box, dtype=np.float32)
    groups, slots = _plan(box)
    key = tuple(slots)
    if key not in _NC_CACHE:
        _NC_CACHE.clear()
        _NC_CACHE[key] = build_kernel(slots)
    nc = _NC_CACHE[key]
    blocks = _prep(img, box, groups, slots)
    in_maps = [{k: np.ascontiguousarray(v) for k, v in b.items()}
               for b in blocks]
    res = run_bass_kernel_spmd(nc, in_maps, list(range(N_CORES)), trace=trace,
                               **(trace_kwargs or {}))
    parts = [res.results[i]["out"] for i in range(N_CORES)]
    return _unpack(parts, groups), res


def kernel(img, box):
    out, _ = _run(img, box, trace=False)
    return out


# revision 24
# speedup vs baseline: 1.1818x; 1.1818x over previous
"""Trainium2 Bass kernel for nn_CropModule: per-sample crop + bilinear resize.

Contract: kernel(img [128,3,480,480] f32, box [128,4] f32) -> [128, 150528] f32.

Strategy (pure data parallel, 16 slots per NeuronCore across 8 cores):
  * The Bass program is specialized to the actual crop sizes at runtime
    (kernel() compiles after seeing the inputs; recompiled whenever the
    crop geometry changes).  The 128 samples are grouped into 16 slots of
    8 (one per core, SPMD) by similar (crop_h, crop_w); each slot's DMA
    rectangles and matmul access patterns are sized to the slot maxima
    CH=max(ch), CW=max(cw) instead of the 240x240 worst case.  That cuts
    HBM traffic ~35% and tensor cycles ~12%, which matters because the
    kernel is DMA-bound (input reads sustain only ~210 GB/s while compute
    runs).
  * Host computes, per sample, the CHxCW crop window and two bilinear
    selection tables RyT [CH,224] / RxT [CW,224] (bilinear weight
    w[i,o] = relu(1 - |i - src(o)|), identical to the reference), packed
    per slot as [R, ty kcy*224 | win (c,yc,x) 3*kcy*CW | tx kcx*224]
    bf16 with R = min(120, max(CH, CW)) partition rows.
  * Device, per slot and channel, two accumulating bf16 matmul passes
    (f32 PSUM accumulate):
        mid[x, oy] = sum_y W[y, x] * RyT[y, oy]      (pass V)
        out[oy, ox] = sum_x mid[x, oy] * RxT[x, ox]  (pass H)
    Pass H of slot u-1 is emitted after the full pass V of slot u so the
    tensor engine never waits on the PSUM->SBUF cast of mid.
  * Host unpacks to [B, 3, 224, 224] f32 (undoing the slot permutation).
    bf16 end-to-end rel err vs the f32 reference is ~8e-3 (gate 2e-2).
"""
from contextlib import ExitStack

import ml_dtypes
import numpy as np

import concourse.bass as bass
import concourse.mybir as mybir
import concourse.tile as tile
from concourse.bass_utils import run_bass_kernel_spmd
from concourse.vector_clock import ScopedClock

IMG = 480
OUT = 224
BATCH = 128
N_CORES = 8
NSAMP = BATCH // N_CORES

OUT_COLS = 3 * 2 * OUT        # 1344: (c, oc, ox)
OUT_BATCH = 2                 # samples per output store group

F32 = mybir.dt.float32
BF16 = mybir.dt.bfloat16
BF16_NP = ml_dtypes.bfloat16


# ------------------------------------------------------------------ geometry

def _crop_dims(box):
    """-> xa, ya, ch, cw (int32 arrays [B])."""
    b = box.astype(np.float32) * np.float32(IMG)
    xa = np.trunc(b[:, 0] - np.float32(0.5) * b[:, 2]).astype(np.int32)
    ya = np.trunc(b[:, 1] - np.float32(0.5) * b[:, 3]).astype(np.int32)
    xb = np.trunc(b[:, 0] + np.float32(0.5) * b[:, 2]).astype(np.int32)
    yb = np.trunc(b[:, 1] + np.float32(0.5) * b[:, 3]).astype(np.int32)
    return xa, ya, (yb - ya).astype(np.int32), (xb - xa).astype(np.int32)


def _plan(box):
    """Group the B samples into NSAMP slots of N_CORES cores each.

    Samples with similar (ch, cw) share a slot so the per-slot maxima are
    tight.  Returns (groups [NSAMP][N_CORES] sample idx, slots [(CH,CW)]).
    """
    _, _, ch, cw = _crop_dims(box)

    def strips(idx, nblk):
        groups = []
        bs = len(idx) // nblk
        for blk in range(nblk):
            bi = idx[blk * bs:(blk + 1) * bs]
            bi = bi[np.argsort(cw[bi], kind="stable")]
            for g in range(bs // N_CORES):
                groups.append(bi[g * N_CORES:(g + 1) * N_CORES])
        return groups

    cands = [strips(np.argsort(ch, kind="stable"), nblk) for nblk in (2, 4, 8)]

    # class-aware: dedicated kcy=1 slots (all ch<=128) and kcx=1 slots
    # (all cw<=128) halve that pass's matmul streams
    sy = np.where(ch <= 128)[0]
    sy = sy[np.argsort(cw[sy], kind="stable")][:len(sy) // N_CORES * N_CORES]
    rest = np.setdiff1d(np.arange(len(ch)), sy)
    sx = rest[cw[rest] <= 128]
    sx = sx[np.argsort(ch[sx], kind="stable")][:len(sx) // N_CORES * N_CORES]
    rem = np.setdiff1d(rest, sx)
    rem = rem[np.argsort(ch[rem], kind="stable")]
    ca = [sy[i * N_CORES:(i + 1) * N_CORES] for i in range(len(sy) // N_CORES)]
    ca += [sx[i * N_CORES:(i + 1) * N_CORES] for i in range(len(sx) // N_CORES)]
    nrem_slots = len(rem) // N_CORES
    for nblk in (2, 4):
        if nrem_slots % nblk == 0 and nrem_slots:
            ca_full = ca + strips(rem, nblk)
            if len(ca_full) * N_CORES == len(ch):
                cands.append(ca_full)
            break

    def cost_of(groups):
        slots = [(int(ch[g].max()), int(cw[g].max())) for g in groups]
        geos = [_Geom(*s) for s in slots]
        cyc = sum(3 * g.kcx * g.kcy * 224 + 2 * g.kcx * 672 for g in geos)
        byt = sum(g.R * g.cols for g in geos)
        return cyc * 3 + byt, slots  # PE-bound: weight cycles over bytes

    best = None
    for groups in cands:
        c, slots = cost_of(groups)
        if best is None or c < best[0]:
            best = (c, groups, slots)
    _, groups, slots = best
    order = list(np.argsort([CH * CW for CH, CW in slots], kind="stable"))
    # the slot with the least pass-H work goes last: the tail between the
    # final matmul and the final store is dominated by H + copies
    hcost = [(_Geom(*slots[i]).kcx, slots[i][0] * slots[i][1]) for i in order]
    last = order.pop(int(np.argmin([h[0] * 1000000 + h[1] for h in hcost])))
    order.append(last)
    groups = [groups[i] for i in order]
    slots = [slots[i] for i in order]
    return groups, slots


def _ceil_div(a, b):
    return -(-a // b)


class _Geom:
    """Per-slot packed-block geometry.  Contraction chunks are split
    EVENLY (not at 120/128) so both chunks have the same row count and the
    packed rectangle has no row padding."""

    def __init__(self, CH, CW):
        self.CH, self.CW = CH, CW
        self.kcy = _ceil_div(CH, 128)
        self.kcx = _ceil_div(CW, 128)
        ry0 = _ceil_div(CH, self.kcy)
        rx0 = _ceil_div(CW, self.kcx)
        self.ry = (ry0, CH - ry0)
        self.rx = (rx0, CW - rx0)
        # a DMA's descriptors are sprayed over (largest divisor of the
        # partition-row count that is <= 16) DMA engines: 120 rows -> 15
        # engines, 82=2*41 -> 2, primes -> 1.  Pad the shipped rows to a
        # multiple of 16 so every input DMA uses all 16 engines.
        self.R = _ceil_div(max(ry0, rx0), 16) * 16
        self.woff = self.kcy * OUT                       # window cols base
        self.txoff = self.woff + 3 * self.kcy * CW       # RxT cols base
        self.cols = self.txoff + self.kcx * OUT


# ---------------------------------------------------------------- host prep

def _axis_tab(ca, cn, w0, rows):
    """Bilinear table [rows, OUT] for one axis of one sample (f32 math as
    in the reference): tab[i, o] = relu(1 - |i - (s(o) + ca - w0)|)."""
    o = np.arange(OUT, dtype=np.float32)
    cnf = np.float32(cn)
    s = np.clip((o + np.float32(0.5)) * cnf / np.float32(OUT)
                - np.float32(0.5), np.float32(0.0), cnf - np.float32(1.0))
    s = s + np.float32(ca - w0)
    i = np.arange(rows, dtype=np.float32)[:, None]
    return np.maximum(np.float32(0.0),
                      np.float32(1.0) - np.abs(i - s[None, :]))


def _prep(img, box, groups, slots):
    """-> per-core input blocks [N_CORES][120, tot_cols] bf16."""
    geos = [_Geom(*s) for s in slots]
    blocks = [{} for _ in range(N_CORES)]
    xa_, ya_, ch_, cw_ = _crop_dims(box)
    for si, (g, geo) in enumerate(zip(groups, geos)):
        CH, CW = geo.CH, geo.CW
        ry0, rx0 = geo.ry[0], geo.rx[0]
        for core, smp in enumerate(g):
            blk = np.zeros((geo.R, geo.cols), dtype=BF16_NP)
            xa, ya = int(xa_[smp]), int(ya_[smp])
            ch, cw = int(ch_[smp]), int(cw_[smp])
            wy0 = min(ya, IMG - CH)
            wx0 = min(xa, IMG - CW)
            ty = _axis_tab(ya, ch, wy0, CH).astype(BF16_NP)
            tx = _axis_tab(xa, cw, wx0, CW).astype(BF16_NP)
            for yc in range(geo.kcy):
                r = geo.ry[yc]
                blk[0:r, yc * OUT:yc * OUT + OUT] = ty[yc * ry0:yc * ry0 + r]
            for xc in range(geo.kcx):
                r = geo.rx[xc]
                c0 = geo.txoff + xc * OUT
                blk[0:r, c0:c0 + OUT] = tx[xc * rx0:xc * rx0 + r]
            win = img[smp, :, wy0:wy0 + CH, wx0:wx0 + CW].astype(BF16_NP)
            for c in range(3):
                for yc in range(geo.kcy):
                    r = geo.ry[yc]
                    c0 = geo.woff + (c * geo.kcy + yc) * CW
                    blk[0:r, c0:c0 + CW] = win[c, yc * ry0:yc * ry0 + r]
            blocks[core][f"inp{si}"] = blk
    return blocks


def _unpack(parts, groups):
    """parts [N_CORES][112, NSAMP*OUT_COLS] bf16 -> [B, 3*224*224] f32."""
    full = np.empty((BATCH, 3, OUT, 2, 112), dtype=np.float32)
    for core in range(N_CORES):
        o = parts[core].reshape(112, NSAMP, 2, 3, OUT)
        o = o.transpose(1, 3, 4, 2, 0).astype(np.float32)  # [slot,c,oy,oc,p]
        for g, grp in enumerate(groups):
            full[grp[core]] = o[g]
    return np.ascontiguousarray(full).reshape(BATCH, -1)


# ------------------------------------------------- walrus wait-limit fixups

class _SplitDrainTileContext(tile.TileContext):
    """The walrus build here rejects instructions carrying several sync
    waits; re-emit the kernel-tail drain's waits as single-wait NoOps,
    spread round-robin across engines so they retire in parallel."""

    def _drain_and_barrier(self, tick_clock, wait_clock):
        nc = self.nc
        probe = nc.sync.nop(nofuse=True, hint="drain_wait_probe")
        wait_clock.add_sem_waits(
            probe.ins, ScopedClock({None: tick_clock.global_clock}))
        si = probe.ins.sync_info
        waits = list(si.on_wait) if si is not None else []
        if si is not None:
            si.on_wait = waits[:1]
        wait_engines = [nc.sync, nc.scalar, nc.vector, nc.tensor]
        for i, w in enumerate(waits[1:]):
            eng = wait_engines[i % len(wait_engines)]
            n = eng.nop(nofuse=True, hint="drain_wait_split")
            n.ins.sync_info = mybir.SyncInfo(on_wait=[w], on_update=[])
        nc.sync.drain()

        nc.all_engine_barrier()
        assert self.sems is not None
        popped = nc._tile_sem_poison_stack.pop()
        assert popped is self._sem_poison
        # Skip the gpsimd range clear + second barrier: the runtime
        # re-initializes kernel semaphores at exec start (validated by
        # back-to-back execs of the same NEFF), and the Q7 wakeup for the
        # clear costs ~5 us of pure tail.


def _split_sync_waits(nc, max_waits=1):
    ctr = 0
    for fn in nc.m.functions:
        for blk in fn.blocks:
            out = []
            for inst in blk.instructions:
                si = getattr(inst, "sync_info", None)
                waits = list(si.on_wait) if si is not None and si.on_wait else []
                if len(waits) > max_waits:
                    for w in waits[:-max_waits]:
                        ctr += 1
                        out.append(mybir.InstNoOp(
                            name=f"wsplit_{ctr}",
                            engine=inst.engine,
                            ins=[], outs=[],
                            sync_info=mybir.SyncInfo(on_wait=[w], on_update=[])))
                    si.on_wait = waits[-max_waits:]
                out.append(inst)
            blk.instructions = out


# ------------------------------------------------------------ device kernel

def build_kernel(slots, n_cores=N_CORES):
    nsamp = len(slots)
    geos = [_Geom(*s) for s in slots]

    nc = bass.Bass("TRN2", target_bir_lowering=False, debug=False,
                   num_devices=n_cores)
    # one DRAM tensor per slot: DMAs that cover a tensor's full partition
    # dim are sprayed across all 16 DMA engines, partition-sliced ones
    # serialize onto a single engine
    inps = [nc.dram_tensor(f"inp{s}", [geos[s].R, geos[s].cols], BF16,
                           kind="ExternalInput") for s in range(nsamp)]
    out = nc.dram_tensor("out", [112, nsamp * OUT_COLS], BF16,
                         kind="ExternalOutput")

    with _SplitDrainTileContext(nc) as tc, ExitStack() as ctx:
        inpp = ctx.enter_context(tc.tile_pool(name="inpp", bufs=nsamp))
        midp = ctx.enter_context(tc.tile_pool(name="midp", bufs=3))
        outp = ctx.enter_context(tc.tile_pool(name="outp", bufs=3))
        midps = ctx.enter_context(tc.tile_pool(name="midps", bufs=3, space="PSUM"))
        abps = ctx.enter_context(tc.tile_pool(name="abps", bufs=3, space="PSUM"))
        cps = ctx.enter_context(tc.tile_pool(name="cps", bufs=2, space="PSUM"))

        pending = None  # (sb, geo, mid3, out_sb, s)

        def emit_h(u):
            sb, geo, mid3, out_sb, s = u
            # mid3 layout (xc, c, oy); moving operand caps at 512 elements,
            # so stream channels (0,1) fused (448 cols) and channel 2 alone
            final = s == nsamp - 1
            for oc in range(2):
                ps_ab = abps.tile([112, 2 * OUT], F32)
                ps_c = cps.tile([112, OUT], F32)
                for xc in range(geo.kcx):
                    xw = geo.rx[xc]
                    t0 = geo.txoff + xc * OUT + oc * 112
                    lhsT = sb[0:xw, t0:t0 + 112]
                    nc.tensor.matmul(
                        ps_ab[:], lhsT=lhsT,
                        rhs=mid3[0:xw, xc * 3 * OUT:xc * 3 * OUT + 2 * OUT],
                        start=(xc == 0), stop=(xc == geo.kcx - 1))
                    nc.tensor.matmul(
                        ps_c[:], lhsT=lhsT,
                        rhs=mid3[0:xw, xc * 3 * OUT + 2 * OUT:
                                 (xc + 1) * 3 * OUT],
                        start=(xc == 0), stop=(xc == geo.kcx - 1))
                ob = (s % OUT_BATCH) * OUT_COLS + oc * 3 * OUT
                if final and oc == 1:
                    # last slot: copy on the idle vector engine (parallel
                    # with scalar's oc=0 copy) and store each half as soon
                    # as it is ready, shrinking the exposed final transfer
                    nc.vector.tensor_copy(out_sb[:, ob:ob + 2 * OUT], ps_ab[:])
                    nc.vector.tensor_copy(
                        out_sb[:, ob + 2 * OUT:ob + 3 * OUT], ps_c[:])
                else:
                    nc.scalar.copy(out=out_sb[:, ob:ob + 2 * OUT], in_=ps_ab[:])
                    nc.scalar.copy(out=out_sb[:, ob + 2 * OUT:ob + 3 * OUT],
                                   in_=ps_c[:])
                if final:
                    # issue the two final stores from different engines so
                    # they go out in parallel
                    eng = nc.sync if oc == 0 else nc.scalar
                    eng.dma_start(
                        out.ap()[:, s * OUT_COLS + oc * 3 * OUT:
                                 s * OUT_COLS + (oc + 1) * 3 * OUT],
                        out_sb[:, ob:ob + 3 * OUT])
            if s == nsamp - 2:
                # store the penultimate slot alone so the final exposed
                # transfer after the last compute is as small as possible
                nc.sync.dma_start(
                    out.ap()[:, s * OUT_COLS:(s + 1) * OUT_COLS],
                    out_sb[:, 0:OUT_COLS])
            elif s != nsamp - 1 and s % OUT_BATCH == OUT_BATCH - 1:
                g0 = s - OUT_BATCH + 1
                nc.scalar.dma_start(
                    out.ap()[:, g0 * OUT_COLS:(g0 + OUT_BATCH) * OUT_COLS],
                    out_sb[:])

        out_sb = None
        for s in range(nsamp):
            geo = geos[s]
            sb = inpp.tile([geo.R, geo.cols], BF16)
            if s == 0:
                # the very first load lands as [RyT + channel-0 window]
                # (all that V(0).c0 gates on), then the rest
                c1 = geo.woff + geo.kcy * geo.CW
                nc.sync.dma_start(sb[:, 0:c1], inps[s].ap()[:, 0:c1])
                nc.sync.dma_start(sb[:, c1:geo.txoff],
                                  inps[s].ap()[:, c1:geo.txoff])
                nc.sync.dma_start(sb[:, geo.txoff:geo.cols],
                                  inps[s].ap()[:, geo.txoff:geo.cols])
            elif s == 1:
                nc.sync.dma_start(sb[:, 0:geo.txoff],
                                  inps[s].ap()[:, 0:geo.txoff])
                nc.sync.dma_start(sb[:, geo.txoff:geo.cols],
                                  inps[s].ap()[:, geo.txoff:geo.cols])
            else:
                nc.sync.dma_start(sb[:], inps[s].ap()[:])
            if s % OUT_BATCH == 0:
                out_sb = outp.tile([112, OUT_BATCH * OUT_COLS], BF16)
            mid3 = midp.tile([128, geo.kcx * 3 * OUT], BF16)
            for c in range(3):
                mid_ps = midps.tile([128, 2 * OUT], F32)
                for xc in range(geo.kcx):
                    xw = geo.rx[xc]
                    for yc in range(geo.kcy):
                        rr = geo.ry[yc]
                        w0 = geo.woff + (c * geo.kcy + yc) * geo.CW \
                            + xc * geo.rx[0]
                        nc.tensor.matmul(
                            mid_ps[0:xw, xc * OUT:(xc + 1) * OUT],
                            lhsT=sb[0:rr, w0:w0 + xw],
                            rhs=sb[0:rr, yc * OUT:(yc + 1) * OUT],
                            start=(yc == 0), stop=(yc == geo.kcy - 1))
                # mid_ps is (xc, oy); scatter the xc halves into the
                # (xc, c, oy) layout of mid3 with one strided-dest copy
                mid3v = mid3[:].rearrange(
                    "p (a c o) -> p a c o", a=geo.kcx, c=3, o=OUT)[:, :, c, :]
                nc.vector.tensor_copy(
                    mid3v, mid_ps[:, 0:geo.kcx * OUT].rearrange(
                        "p (a o) -> p a o", a=geo.kcx, o=OUT))
            # emit H(s-1) only now: the full V(s) sits between the c2 cast
            # of slot s-1 (vector) and the H matmuls that consume it, so
            # the tensor engine never waits on the cast
            if pending is not None:
                emit_h(pending)
            pending = (sb, geo, mid3, out_sb, s)
        emit_h(pending)
    _split_sync_waits(nc)
    return nc


_NC_CACHE = {}


def _run(img, box, trace=False, trace_kwargs=None):
    img = np.asarray(img, dtype=np.float32)
    box = np.asarray(box, dtype=np.float32)
    groups, slots = _plan(box)
    key = tuple(slots)
    if key not in _NC_CACHE:
        _NC_CACHE.clear()
        _NC_CACHE[key] = build_kernel(slots)
    nc = _NC_CACHE[key]
    blocks = _prep(img, box, groups, slots)
    in_maps = [{k: np.ascontiguousarray(v) for k, v in b.items()}
               for b in blocks]
    res = run_bass_kernel_spmd(nc, in_maps, list(range(N_CORES)), trace=trace,
                               **(trace_kwargs or {}))
    parts = [res.results[i]["out"] for i in range(N_CORES)]
    return _unpack(parts, groups), res


def kernel(img, box):
    out, _ = _run(img, box, trace=False)
    return out
